# revision 87
# baseline (speedup 1.0000x reference)
"""AdaptiveNeuromorphicNetwork Trainium2 kernel (8 NeuronCores, SPMD).

Sharding: output neurons H=2048 split 256/core (H-shard) -> the LIF scan,
spike-rate mean (over batch) and homeostatic threshold update are fully local
per core; zero collectives. input_spikes are replicated (each core streams all
of them through the TensorEngine against its weight column shard).

Matmul scheme ("fp16dc"): single fp16-weight pass with the MOVING operand
being the fp8 spikes directly (0/1 exact in fp8; the PE runs mixed
fp16-stationary x fp8-moving at 1 cyc/row). The dropped fp8 residual plane is
compensated by a host-computed DC correction: the time-mean residual current
c[b,h] = -sum_i (fp16(w)-w)[i,h] * mean_t s[b,i,t], which the double-EMA
dynamics amplify ~4x over the white part. c is injected into the same PSUM
accumulation group as a 64-row "rider" matmul (bf16 weights = c per batch
row, fp8 moving identity-per-batch pattern). CPU-validated rel err 0.0165
(gate 2e-2); fp16-subnormal weights are zeroed host-side so host E matches
the device exactly.

Pipeline: columns are TIME-MAJOR and each chunk's matmuls run as per-
time-block PSUM groups (<=8 steps = 512 cols) that stop independently, so
the scalar-engine evacuation of block N overlaps the matmuls of block N+1
and the DVE scan chases the PE with ~zero handoff. The LIF scan keeps the
entire serial chain on the DVE (i_syn STT, fused LIF_S/LIF_V custom ops,
and a 2-op threshold update using exponentially-rescaled partial sums:
P += c*0.99^-t * rs ; nT += 0.99^t * P + Rd_t, with the deterministic
drift Rd_t folded into per-instruction immediates) -- no cross-engine
dependencies anywhere in the steady-state loop, so nothing parks and the
step cadence is pure engine time (828 ns/step: TSP 194 + 4x LIF 127 +
2 threshold ops 63; every same-engine sem edge is hidden by emitting the
i_syn STT two steps ahead and double-buffering nT by step parity so SACC
never WARs LIF_V).

Further scheduling: hi-pass weights are scaled by 2^13 (exact) so the
fp8-DoubleRow rider stays in fp8's normal range; the evacuation descales.
DMA order on one sync queue [w(k0-3), spk0(k0-3), spk0(k4-15), w(k4-15),
spk1, rider, thr, spk2..] feeds the tapered chunk schedule in CHUNKS;
out-DMAs ride the sync queue 3 chunks delayed so they never block the
evacuations' counting semaphore (final chunk per-step so setup overlaps
the last scan step); ~28 warmup matmuls on memset garbage ramp the PE
p-state before the first real matmul; tiny end-chunks use a dedicated
PSUM bank (psume) outside the main rotation; the dead v'/threshold ops
of the very last step are skipped. PSUM time-blocks are 4 steps (fine-
grained evac feed). TimelineSim: 129368 ns (baseline
166631 -> 1.29x).
"""
import numpy as np

import concourse.bass as bass
import concourse.tile as tile
from concourse import bacc, mybir
from concourse.bass_utils import run_bass_kernel_spmd

B, I, H, T = 64, 2048, 2048, 128
NCORES = 8
HL = H // NCORES            # 256 output neurons per core
KT = I // 128               # 16 contraction tiles
# tapered at both ends: small first chunks let the PE start before the bulk
# spike DMA lands; small last chunks keep the serial LIF-scan tail short
CHUNKS = [4, 8, 16, 16, 16, 16, 16, 16, 14, 2, 2, 2]
NCH = len(CHUNKS)
assert sum(CHUNKS) == T
DT = 0.001
TCS = sorted(set(CHUNKS))    # distinct chunk lengths (rider pattern per TC)
SB = 256 // B                # steps per PSUM time-block (4)


def _tblocks(tc):
    """Chop a chunk of tc steps into time-blocks of <=SB steps."""
    out = []
    a = 0
    while a < tc:
        b = min(a + SB, tc)
        out.append((a, b))
        a = b
    return out


_F32 = mybir.dt.float32
_ALU = mybir.AluOpType

# ---- custom fused DVE ops for the LIF step ----
import operator as _op

import concourse.dve_ops as _dve_ops
from concourse.dve_ops import DveOp as _DveOp
from concourse.dve_spec import (Spec as _Spec, Src0 as _Src0, Src1 as _Src1,
                                C0 as _C0, C1 as _C1, C2 as _C2, Zero as _Zero,
                                lower as _lower, _has_src1)
from concourse.dve_table_gen import dve_ver_for as _dve_ver_for
from concourse.dve_uop import DveOpSpec as _DveOpSpec


def _register_dve(name, spec):
    if name in _dve_ops._SUB_OPCODE_FOR_NAME:
        for o in _dve_ops.OPS:
            if o.name == name:
                return o
    ver = _dve_ver_for("TRN2")
    opcode = max(_dve_ops._SUB_OPCODE_FOR_NAME.values()) + 1
    assert opcode < 0x20
    sha = _DveOpSpec(name=name, opcode=opcode, uops=_lower(spec, ver=ver),
                     rd1_en=_has_src1(spec)).sha(ver)
    dop = _DveOp(name, spec, subdim=False, uops_sha={ver: sha})
    _dve_ops.OPS.append(dop)
    _dve_ops.CUSTOM_DVE_SPECS[name] = spec
    _dve_ops._SUB_OPCODE_FOR_NAME[name] = opcode
    return dop


def _lif_s_ref(in0, in1, s0, s1, imm2):
    P = in1.astype(np.float32) * s0 + in0
    s = (P + s1 >= 0).astype(np.float32)
    return s, s.reshape(s.shape[0], -1).sum(axis=-1, keepdims=True)


def _lif_v_ref(in0, in1, s0, s1, imm2):
    P = in1.astype(np.float32) * s0 + in0
    s = (P + s1 >= 0).astype(np.float32)
    return P + s * s1


def _sacc_ref(in0, in1, s0, s1, imm2):
    return in0.astype(np.float32) * s0 + in1 + imm2


# s = ((v*a_mem + isyn) + negThr) >= 0 ; accum = sum(s) over free dim
_P = _Src1 * _C0 + _Src0
LIF_S = _register_dve("LIF_S", _Spec(body=(_P + _C1) >= _Zero,
                                     accum=_op.add, reference=_lif_s_ref))
# v' = P + ((P + negThr) >= 0) * negThr
LIF_V = _register_dve("LIF_V", _Spec(body=_P + ((_P + _C1) >= _Zero) * _C1,
                                     reference=_lif_v_ref))
# nT += 0.99^t * P + Rd_t  (threshold integrator step)
SACC = _register_dve("SACC", _Spec(body=_Src0 * _C0 + _Src1 + _C2,
                                   reference=_sacc_ref))


def _build(a_mem, a_syn, lr, tgt):
    """Build + compile the per-core Bass graph (same graph on all 8 cores)."""
    nc = bacc.Bacc("TRN2", target_bir_lowering=False, debug=False,
                   num_devices=NCORES)
    # spikes: flat, per chunk c: KT blocks of [128, tc*B] (time-major), fp8
    spk8 = nc.dram_tensor("spk8", [KT * 128, T * B], mybir.dt.float8e4,
                          kind="ExternalInput").ap()
    # weights: [i128, (k,ht,h)] fp16
    wgt = nc.dram_tensor("wgt", [128, KT * 2 * 128], mybir.dt.float16,
                         kind="ExternalInput").ap()
    # rider (fp8 DoubleRow, k=64 as 32x2): moving pattern
    # delta_{ko*32+p, b}, one (ko,n) block per distinct TC
    rid = nc.dram_tensor("rid", [32, 2 * sum(TCS) * B], mybir.dt.float8e4,
                         kind="ExternalInput").ap()
    # rider weights: DC correction * 2^13, [32, (ht, ko, h)]
    crd = nc.dram_tensor("crd", [32, 2 * 2 * 128], mybir.dt.float8e4,
                         kind="ExternalInput").ap()
    nt0 = nc.dram_tensor("nt0", [128, 2], _F32, kind="ExternalInput").ap()
    odt = mybir.dt.bfloat16
    out = nc.dram_tensor("out", [128, T * 128], odt, kind="ExternalOutput").ap()

    a_mem, a_syn, lr, tgt = float(a_mem), float(a_syn), float(lr), float(tgt)
    c_ema = float(np.float32(-lr / 6400.0))
    k1 = float(np.float32(0.01 * lr * tgt))
    r0 = float(np.float32(lr * tgt))
    # threshold integrator scalars per step:
    #   P_t = P_{t-1} + (c*0.99^-t) * rs_t ; nT += 0.99^t * P_t + Rd_t
    #   Rd_t = 0.99^{t+1} r0 + k1 (1 - 0.99^{t+1}) / 0.01
    sc_p = [float(c_ema * 0.99 ** (-t)) for t in range(T)]
    sc_s = [float(0.99 ** t) for t in range(T)]
    rd = [float(0.99 ** (t + 1) * r0 + k1 * (1.0 - 0.99 ** (t + 1)) / 0.01)
          for t in range(T)]

    with tile.TileContext(nc) as tc:
        with tc.tile_pool(name="wpool", bufs=1) as wpool, \
             tc.tile_pool(name="state", bufs=1) as state, \
             tc.tile_pool(name="spkp", bufs=2) as spkp, \
             tc.tile_pool(name="psum", bufs=2, space="PSUM") as psum, \
             tc.tile_pool(name="psume", bufs=1, space="PSUM") as psume, \
             tc.tile_pool(name="wev", bufs=12) as wev, \
             tc.tile_pool(name="accp", bufs=6) as accp, \
             tc.tile_pool(name="tmp", bufs=8) as tmp:

            # ---- persistent tiles ----
            # weights split into 4 k-group tiles; group 0 first on the sync
            # queue, the rest stream behind chunk 0's spikes
            KG = 4
            wsbk = [wpool.tile([128, KG * 2 * 128], mybir.dt.float16,
                               tag="wsb0", name="wsb0")]
            wsbr = wpool.tile([128, 3 * KG * 2 * 128], mybir.dt.float16,
                              tag="wsbr", name="wsbr")
            wsbk += [wsbr[:, (g - 1) * KG * 2 * 128:g * KG * 2 * 128]
                     for g in range(1, KT // KG)]
            rsb = wpool.tile([32, 2 * sum(TCS) * B], mybir.dt.float8e4,
                             tag="rsb")
            roff = {tc_: 2 * B * sum(t for t in TCS if t < tc_)
                    for tc_ in TCS}
            csb = wpool.tile([32, 2 * 2 * 128], mybir.dt.float8e4, tag="csb")
            # nT double-buffered by step parity: SACC(t) writes nT[(t+1)%2]
            # while LIF_V(t) still reads nT[t%2] -> no WAR, SACC can run early
            nTs = [state.tile([128, 2], _F32, tag=f"nT{i}", name=f"nT{i}")
                   for i in range(2)]
            Pt = state.tile([128, 2], _F32, tag="Pt")
            nc.vector.memset(Pt[:], 0.0)
            vst = [state.tile([128, 128], _F32, tag=f"v{i}", name=f"v{i}")
                   for i in range(2)]
            ist = [state.tile([128, 128], _F32, tag=f"i{i}", name=f"isyn{i}")
                   for i in range(4)]
            nc.vector.memset(vst[0][:], 0.0)
            nc.vector.memset(ist[0][:], 0.0)

            # prefetch spikes for chunks 0-1 ahead of weight groups 1-3 on
            # the sync queue (DMA order = issue order; chunk 0's k>=4
            # matmuls wait briefly but the spike stream never starves)
            def spk_dma(c, t0):
                TC = CHUNKS[c]
                spk_t = spkp.tile([128, KT * TC * B], mybir.dt.float8e4,
                                  tag="spk", name=f"spk_c{c}")
                nc.sync.dma_start(
                    spk_t[:].rearrange("p (k n) -> p k n", k=KT),
                    spk8.rearrange("(k p) n -> p k n",
                                   k=KT)[:, :, B * t0:B * t0 + TC * B])
                return spk_t

            # single sync-queue order (DMA_ENGINES is serial, order = issue
            # order): w0 s0 w-rest s1 rider/threshold s2 ...
            nc.sync.dma_start(wsbk[0][:], wgt[:, 0:KG * 2 * 128])
            # chunk 0's spikes split k0-3 / k4-15 so the very first matmul
            # waits only a quarter of the chunk-0 spike transfer
            TC0 = CHUNKS[0]
            s0A = spkp.tile([128, 4 * TC0 * B], mybir.dt.float8e4,
                            tag="spkA", name="spkA")
            nc.sync.dma_start(
                s0A[:].rearrange("p (k n) -> p k n", k=4),
                spk8.rearrange("(k p) n -> p k n", k=KT)[:, 0:4, 0:TC0 * B])
            s0B = spkp.tile([128, 12 * TC0 * B], mybir.dt.float8e4,
                            tag="spkB", name="spkB")
            nc.sync.dma_start(
                s0B[:].rearrange("p (k n) -> p k n", k=12),
                spk8.rearrange("(k p) n -> p k n", k=KT)[:, 4:16, 0:TC0 * B])
            pre_spk = [None]
            nc.sync.dma_start(wsbr[:], wgt[:, KG * 2 * 128:])
            pre_spk.append(spk_dma(1, CHUNKS[0]))
            nc.sync.dma_start(rsb[:], rid[:])
            nc.sync.dma_start(csb[:], crd[:])
            nc.sync.dma_start(nTs[0][:], nt0[:])

            # PE warmup: dummy matmuls on memset garbage keep the PE busy
            # from ~0.7us so the p-state ramp completes before real work;
            # they write a psume-bank tile that chunk 0 resets afterwards
            dumw = wpool.tile([128, 128], mybir.dt.float16, tag="dumw")
            nc.gpsimd.memset(dumw[:], 0.0)
            dumr = wpool.tile([128, 512], mybir.dt.float16, tag="dumr")
            nc.gpsimd.memset(dumr[:], 0.0)
            dps = psume.tile([128, 256], _F32, tag="pse0", name="dumps")
            for _ in range(28):
                nc.tensor.matmul(dps[:], dumw[:], dumr[:, 0:256],
                                 start=True, stop=True)

            # global step -> (w3 view, local index); filled as evacs are
            # emitted one chunk ahead of the scan
            w3map = []
            accs = []       # (acc tile, chunk start, TC) per chunk
            cstart = [0]
            for tc_ in CHUNKS:
                cstart.append(cstart[-1] + tc_)

            def emit_mm_evac(c):
                TC = CHUNKS[c]
                t0 = cstart[c]
                spk_t = pre_spk[c] if c < len(pre_spk) else spk_dma(c, t0)
                pss = []
                for (ta, tb) in _tblocks(TC):
                    nb = tb - ta
                    # tiny end-chunks use a dedicated PSUM bank so they
                    # don't WAR against the main double-buffer rotation
                    pool = psum if TC > 4 else psume
                    ps = [pool.tile([128, nb * B], _F32, tag=f"ps{ht}"
                                    if TC > 4 else f"pse{ht}",
                                    name=f"ps{c}_{ta}_{ht}")
                          for ht in range(2)]
                    pss.append(ps)
                    for k in range(KT):
                        for ht in range(2):
                            lhsT = wsbk[k // KG][:, ((k % KG) * 2 + ht) * 128:
                                                 ((k % KG) * 2 + ht + 1) * 128]
                            if c == 0 and k < 4:
                                rhs = s0A[:, k * TC * B + ta * B:
                                          k * TC * B + tb * B]
                            elif c == 0:
                                rhs = s0B[:, (k - 4) * TC * B + ta * B:
                                          (k - 4) * TC * B + tb * B]
                            else:
                                rhs = spk_t[:, k * TC * B + ta * B:
                                            k * TC * B + tb * B]
                            nc.tensor.matmul(
                                ps[ht][:], lhsT, rhs,
                                start=(k == 0), stop=False)
                    # DC-correction rider (fp8 DoubleRow) carries the stop
                    r8 = rsb[:, roff[TC]:roff[TC] + 2 * TC * B].rearrange(
                        "p (ko n) -> p ko n", ko=2)[:, :, ta * B:tb * B]
                    for ht in range(2):
                        l8 = csb[:, ht * 256:(ht + 1) * 256].rearrange(
                            "p (ko h) -> p ko h", ko=2)
                        nc.tensor.matmul(
                            ps[ht][:], l8, r8,
                            start=False, stop=True,
                            perf_mode=mybir.MatmulPerfMode.DoubleRow)
                # ---- evacuate PSUM -> SBUF (scalar engine) ----
                with tc.high_priority():
                    for bi, (ta, tb) in enumerate(_tblocks(TC)):
                        nb = tb - ta
                        wt = wev.tile([128, 2 * nb * B], _F32, tag="wt",
                                      name=f"wt{c}_{ta}")
                        for ht in range(2):
                            nc.scalar.activation(
                                wt[:, ht * nb * B:(ht + 1) * nb * B],
                                pss[bi][ht][:],
                                mybir.ActivationFunctionType.Copy,
                                bias=0.0, scale=float(2.0 ** -13))
                        w3 = wt[:].rearrange("p (g t b) -> p g t b",
                                             g=2, b=B)
                        for tl in range(ta, tb):
                            w3map.append((w3, tl - ta))
                accs.append(accp.tile([128, TC * 128], odt, tag="acc",
                                      name=f"acc{c}"))

            def emit_isyn(t):
                i3o = ist[t % 4][:].rearrange("p (h b) -> p h b", h=2)
                i3n = ist[(t + 1) % 4][:].rearrange("p (h b) -> p h b", h=2)
                w3, j = w3map[t]
                nc.vector.scalar_tensor_tensor(
                    i3n, i3o, a_syn, w3[:, :, j, :],
                    op0=_ALU.mult, op1=_ALU.add)

            def emit_scan_chunk(c):
                # per-step order [S0,S1,P,SACC,V0,V1,TSP(t+2)]: the i_syn
                # STT for step t+2 is emitted at step t (possibly into the
                # next chunk's just-evacuated weights), so every sem edge
                # hides under independent engine work
                TC = CHUNKS[c]
                t0 = cstart[c]
                acc = accs[c]
                if c == 0:
                    emit_isyn(0)
                    emit_isyn(1)
                for tl in range(TC):
                    t = t0 + tl
                    inew = ist[(t + 1) % 4]
                    vold, vnew = vst[t % 2], vst[(t + 1) % 2]
                    nTc, nTn = nTs[t % 2], nTs[(t + 1) % 2]
                    rs = tmp.tile([128, 2], _F32, tag="rs")
                    for ht in range(2):
                        sl = slice(ht * B, (ht + 1) * B)
                        s_out = acc[:, tl * 128 + ht * B:
                                    tl * 128 + (ht + 1) * B]
                        nc.vector._custom_dve(
                            LIF_S, out=s_out, in0=inew[:, sl],
                            in1=vold[:, sl],
                            s0=a_mem, s1=nTc[:, ht:ht + 1],
                            accum_out=rs[:, ht:ht + 1])
                    if t == T - 1:
                        continue  # v'/threshold of the last step are unused
                    # threshold: P += (c*0.99^-t)*rs ; nT' = 0.99^t*P+nT+Rd
                    nc.vector.scalar_tensor_tensor(
                        Pt[:], rs[:], sc_p[t], Pt[:],
                        op0=_ALU.mult, op1=_ALU.add)
                    nc.vector._custom_dve(
                        SACC, out=nTn[:], in0=Pt[:], in1=nTc[:],
                        s0=sc_s[t], imm2=rd[t])
                    for ht in range(2):
                        sl = slice(ht * B, (ht + 1) * B)
                        nc.vector._custom_dve(
                            LIF_V, out=vnew[:, sl], in0=inew[:, sl],
                            in1=vold[:, sl], s0=a_mem, s1=nTc[:, ht:ht + 1])
                    if t + 2 < len(w3map):
                        emit_isyn(t + 2)

            def emit_out(c):
                TC = CHUNKS[c]
                t0 = cstart[c]
                if c == NCH - 1:
                    # final chunk: per-step DMAs so the first piece's queue
                    # setup overlaps the last scan step
                    for tl in range(TC):
                        nc.sync.dma_start(
                            out[:, (t0 + tl) * 128:(t0 + tl + 1) * 128],
                            accs[c][:, tl * 128:(tl + 1) * 128])
                else:
                    nc.sync.dma_start(out[:, t0 * 128:(t0 + TC) * 128],
                                      accs[c][:])

            # chunk-level software pipeline: mm+evac(c) before scan(c-1);
            # out-DMA(c-1) emitted after evac(c+1) so it never delays an
            # evacuation in the in-order Act queue
            emit_mm_evac(0)
            for c in range(NCH):
                if c + 1 < NCH:
                    emit_mm_evac(c + 1)
                if c >= 3:
                    emit_out(c - 3)
                emit_scan_chunk(c)
            for c in range(NCH - 3, NCH):
                emit_out(c)
    nc.compile()
    return nc


_CACHE = {}


def _get_nc(a_mem, a_syn, lr, tgt):
    key = (tuple(CHUNKS), float(a_mem), float(a_syn), float(lr), float(tgt))
    if key not in _CACHE:
        _CACHE[key] = _build(a_mem, a_syn, lr, tgt)
    return _CACHE[key]


def kernel(input_spikes, weight, synaptic_strength, threshold,
           tau_mem, tau_syn, target_rate, homeostatic_lr):
    import ml_dtypes
    spikes = np.asarray(input_spikes, dtype=np.float32)
    w_eff = (np.asarray(weight, dtype=np.float32)
             * np.asarray(synaptic_strength, dtype=np.float32))
    thr = np.asarray(threshold, dtype=np.float32)
    tau_m = np.float32(tau_mem)
    tau_s = np.float32(tau_syn)
    tgt = np.float32(target_rate)
    lr = np.float32(homeostatic_lr)
    a_mem = np.float32(np.exp(np.float64(np.float32(-DT) / tau_m)))
    a_syn = np.float32(np.exp(np.float64(np.float32(-DT) / tau_s)))

    nc = _get_nc(a_mem, a_syn, lr, tgt)

    # quantize weights; zero fp16 subnormals so host E matches device exactly
    w16 = w_eff.astype(np.float16)
    w16[np.abs(w16.astype(np.float32)) < np.float32(2.0 ** -14)] = \
        np.float16(0.0)
    E = w16.astype(np.float32) - w_eff                      # [I, H]
    sbar = spikes.mean(axis=2)                              # [B, I], exact
    c_full = -(sbar @ E)                                    # [B, H] fp32

    # spikes [B,I,T] -> [I, T*B] chunk-blocked, TIME-MAJOR inside chunks
    sIT = spikes.transpose(1, 0, 2)      # [I, B, T]
    pieces = []
    t0 = 0
    for tc_ in CHUNKS:
        pieces.append(np.ascontiguousarray(
            sIT[:, :, t0:t0 + tc_].transpose(0, 2, 1)).reshape(I, tc_ * B))
        t0 += tc_
    spk8_prep = np.ascontiguousarray(
        np.concatenate(pieces, axis=1)).astype(ml_dtypes.float8_e4m3)

    # rider moving data (DoubleRow, k=64 as 32x2): per distinct TC a
    # [32, (ko, t*B+b)] block with rid[p, ko, t*B+b] = (ko*32+p == b)
    piecesd = []
    eye = np.eye(B, dtype=np.float32)            # [b', b]
    for tc_ in sorted(set(CHUNKS)):
        pat = np.kron(np.ones((1, tc_), np.float32), eye)   # [64, tc*B]
        piecesd.append(pat.reshape(2, 32, tc_ * B).transpose(
            1, 0, 2).reshape(32, 2 * tc_ * B))
    rid_prep = np.ascontiguousarray(
        np.concatenate(piecesd, axis=1)).astype(ml_dtypes.float8_e4m3)

    WSC = np.float32(2.0 ** 13)   # psum scale (descaled in the evacuation)
    in_maps = []
    for core in range(NCORES):
        shard16 = w16[:, core * HL:(core + 1) * HL]          # [I, 256] fp16
        wk = (shard16.astype(np.float32) * WSC).astype(np.float16)
        wk = wk.reshape(KT, 128, 2, 128).transpose(0, 2, 1, 3)
        wk = np.ascontiguousarray(wk.transpose(2, 0, 1, 3)
                                  ).reshape(128, KT * 2 * 128)  # [i,(k,ht,h)]
        nt0 = np.ascontiguousarray(
            -thr[core * HL:(core + 1) * HL].reshape(2, 128).T)
        cs = c_full[:, core * HL:(core + 1) * HL] * WSC      # [64, 256]
        # [b=ko*32+p, (ht,h)] -> [32, (ht, ko, h)]
        c8 = cs.reshape(2, 32, 2, 128).transpose(1, 2, 0, 3).reshape(32, 512)
        im = {"wgt": wk, "nt0": nt0, "spk8": spk8_prep,
              "crd": np.ascontiguousarray(c8.astype(ml_dtypes.float8_e4m3)),
              "rid": rid_prep}
        in_maps.append(im)

    res = run_bass_kernel_spmd(nc, in_maps, core_ids=list(range(NCORES)),
                               trace=False)
    kernel.last_result = res

    outs = []
    for core in range(NCORES):
        o = res.results[core]["out"].astype(np.float32).reshape(128, T, 2, B)
        outs.append(o.transpose(3, 2, 0, 1).reshape(B, HL, T))
    return np.ascontiguousarray(np.concatenate(outs, axis=1))


# revision 88
# speedup vs baseline: 1.0019x; 1.0019x over previous
"""AdaptiveNeuromorphicNetwork Trainium2 kernel (8 NeuronCores, SPMD).

Sharding: output neurons H=2048 split 256/core (H-shard) -> the LIF scan,
spike-rate mean (over batch) and homeostatic threshold update are fully local
per core; zero collectives. input_spikes are replicated (each core streams all
of them through the TensorEngine against its weight column shard).

Matmul scheme ("fp16dc"): single fp16-weight pass with the MOVING operand
being the fp8 spikes directly (0/1 exact in fp8; the PE runs mixed
fp16-stationary x fp8-moving at 1 cyc/row). The dropped fp8 residual plane is
compensated by a host-computed DC correction: the time-mean residual current
c[b,h] = -sum_i (fp16(w)-w)[i,h] * mean_t s[b,i,t], which the double-EMA
dynamics amplify ~4x over the white part. c is injected into the same PSUM
accumulation group as a 64-row "rider" matmul (bf16 weights = c per batch
row, fp8 moving identity-per-batch pattern). CPU-validated rel err 0.0165
(gate 2e-2); fp16-subnormal weights are zeroed host-side so host E matches
the device exactly.

Pipeline: columns are TIME-MAJOR and each chunk's matmuls run as per-
time-block PSUM groups (<=8 steps = 512 cols) that stop independently, so
the scalar-engine evacuation of block N overlaps the matmuls of block N+1
and the DVE scan chases the PE with ~zero handoff. The LIF scan keeps the
entire serial chain on the DVE (i_syn STT, fused LIF_S/LIF_V custom ops,
and a 2-op threshold update using exponentially-rescaled partial sums:
P += c*0.99^-t * rs ; nT += 0.99^t * P + Rd_t, with the deterministic
drift Rd_t folded into per-instruction immediates) -- no cross-engine
dependencies anywhere in the steady-state loop, so nothing parks and the
step cadence is pure engine time (828 ns/step: TSP 194 + 4x LIF 127 +
2 threshold ops 63; every same-engine sem edge is hidden by emitting the
i_syn STT two steps ahead and double-buffering nT by step parity so SACC
never WARs LIF_V).

Further scheduling: hi-pass weights are scaled by 2^13 (exact) so the
fp8-DoubleRow rider stays in fp8's normal range; the evacuation descales.
DMA order on one sync queue [w(k0-3), spk0(k0-3), spk0(k4-15), w(k4-15),
spk1, rider, thr, spk2..] feeds the tapered chunk schedule in CHUNKS;
out-DMAs ride the sync queue 3 chunks delayed so they never block the
evacuations' counting semaphore (final chunk per-step so setup overlaps
the last scan step); ~28 warmup matmuls on memset garbage ramp the PE
p-state before the first real matmul; tiny end-chunks use a dedicated
PSUM bank (psume) outside the main rotation; the dead v'/threshold ops
of the very last step are skipped. PSUM time-blocks are 4 steps (fine-
grained evac feed). TimelineSim: 129368 ns (baseline
166631 -> 1.29x).
"""
import numpy as np

import concourse.bass as bass
import concourse.tile as tile
from concourse import bacc, mybir
from concourse.bass_utils import run_bass_kernel_spmd

B, I, H, T = 64, 2048, 2048, 128
NCORES = 8
HL = H // NCORES            # 256 output neurons per core
KT = I // 128               # 16 contraction tiles
# tapered at both ends: small first chunks let the PE start before the bulk
# spike DMA lands; small last chunks keep the serial LIF-scan tail short
CHUNKS = [4, 8, 16, 16, 16, 16, 16, 16, 14, 2, 2, 2]
NCH = len(CHUNKS)
assert sum(CHUNKS) == T
DT = 0.001
TCS = sorted(set(CHUNKS))    # distinct chunk lengths (rider pattern per TC)
SB = 256 // B                # steps per PSUM time-block (4)


def _tblocks(tc):
    """Chop a chunk of tc steps into time-blocks of <=SB steps."""
    out = []
    a = 0
    while a < tc:
        b = min(a + SB, tc)
        out.append((a, b))
        a = b
    return out


_F32 = mybir.dt.float32
_ALU = mybir.AluOpType

# ---- custom fused DVE ops for the LIF step ----
import operator as _op

import concourse.dve_ops as _dve_ops
from concourse.dve_ops import DveOp as _DveOp
from concourse.dve_spec import (Spec as _Spec, Src0 as _Src0, Src1 as _Src1,
                                C0 as _C0, C1 as _C1, C2 as _C2, Zero as _Zero,
                                lower as _lower, _has_src1)
from concourse.dve_table_gen import dve_ver_for as _dve_ver_for
from concourse.dve_uop import DveOpSpec as _DveOpSpec


def _register_dve(name, spec):
    if name in _dve_ops._SUB_OPCODE_FOR_NAME:
        for o in _dve_ops.OPS:
            if o.name == name:
                return o
    ver = _dve_ver_for("TRN2")
    opcode = max(_dve_ops._SUB_OPCODE_FOR_NAME.values()) + 1
    assert opcode < 0x20
    sha = _DveOpSpec(name=name, opcode=opcode, uops=_lower(spec, ver=ver),
                     rd1_en=_has_src1(spec)).sha(ver)
    dop = _DveOp(name, spec, subdim=False, uops_sha={ver: sha})
    _dve_ops.OPS.append(dop)
    _dve_ops.CUSTOM_DVE_SPECS[name] = spec
    _dve_ops._SUB_OPCODE_FOR_NAME[name] = opcode
    return dop


def _lif_s_ref(in0, in1, s0, s1, imm2):
    P = in1.astype(np.float32) * s0 + in0
    s = (P + s1 >= 0).astype(np.float32)
    return s, s.reshape(s.shape[0], -1).sum(axis=-1, keepdims=True)


def _lif_v_ref(in0, in1, s0, s1, imm2):
    P = in1.astype(np.float32) * s0 + in0
    s = (P + s1 >= 0).astype(np.float32)
    return P + s * s1


def _sacc_ref(in0, in1, s0, s1, imm2):
    return in0.astype(np.float32) * s0 + in1 + imm2


# s = ((v*a_mem + isyn) + negThr) >= 0 ; accum = sum(s) over free dim
_P = _Src1 * _C0 + _Src0
LIF_S = _register_dve("LIF_S", _Spec(body=(_P + _C1) >= _Zero,
                                     accum=_op.add, reference=_lif_s_ref))
# v' = P + ((P + negThr) >= 0) * negThr
LIF_V = _register_dve("LIF_V", _Spec(body=_P + ((_P + _C1) >= _Zero) * _C1,
                                     reference=_lif_v_ref))
# nT += 0.99^t * P + Rd_t  (threshold integrator step)
SACC = _register_dve("SACC", _Spec(body=_Src0 * _C0 + _Src1 + _C2,
                                   reference=_sacc_ref))


def _build(a_mem, a_syn, lr, tgt):
    """Build + compile the per-core Bass graph (same graph on all 8 cores)."""
    nc = bacc.Bacc("TRN2", target_bir_lowering=False, debug=False,
                   num_devices=NCORES)
    # spikes: flat, per chunk c: KT blocks of [128, tc*B] (time-major), fp8
    spk8 = nc.dram_tensor("spk8", [KT * 128, T * B], mybir.dt.float8e4,
                          kind="ExternalInput").ap()
    # weights: [i128, (k,ht,h)] fp16
    wgt = nc.dram_tensor("wgt", [128, KT * 2 * 128], mybir.dt.float16,
                         kind="ExternalInput").ap()
    # rider (fp8 DoubleRow, k=64 as 32x2): moving pattern
    # delta_{ko*32+p, b}, one (ko,n) block per distinct TC
    rid = nc.dram_tensor("rid", [32, 2 * sum(TCS) * B], mybir.dt.float8e4,
                         kind="ExternalInput").ap()
    # rider weights: DC correction * 2^13, [32, (ht, ko, h)]
    crd = nc.dram_tensor("crd", [32, 2 * 2 * 128], mybir.dt.float8e4,
                         kind="ExternalInput").ap()
    nt0 = nc.dram_tensor("nt0", [128, 2], _F32, kind="ExternalInput").ap()
    odt = mybir.dt.bfloat16
    out = nc.dram_tensor("out", [128, T * 128], odt, kind="ExternalOutput").ap()

    a_mem, a_syn, lr, tgt = float(a_mem), float(a_syn), float(lr), float(tgt)
    c_ema = float(np.float32(-lr / 6400.0))
    k1 = float(np.float32(0.01 * lr * tgt))
    r0 = float(np.float32(lr * tgt))
    # threshold integrator scalars per step:
    #   P_t = P_{t-1} + (c*0.99^-t) * rs_t ; nT += 0.99^t * P_t + Rd_t
    #   Rd_t = 0.99^{t+1} r0 + k1 (1 - 0.99^{t+1}) / 0.01
    sc_p = [float(c_ema * 0.99 ** (-t)) for t in range(T)]
    sc_s = [float(0.99 ** t) for t in range(T)]
    rd = [float(0.99 ** (t + 1) * r0 + k1 * (1.0 - 0.99 ** (t + 1)) / 0.01)
          for t in range(T)]

    with tile.TileContext(nc) as tc:
        with tc.tile_pool(name="wpool", bufs=1) as wpool, \
             tc.tile_pool(name="state", bufs=1) as state, \
             tc.tile_pool(name="spkp", bufs=2) as spkp, \
             tc.tile_pool(name="psum", bufs=2, space="PSUM") as psum, \
             tc.tile_pool(name="psume", bufs=2, space="PSUM") as psume, \
             tc.tile_pool(name="wev", bufs=12) as wev, \
             tc.tile_pool(name="accp", bufs=6) as accp, \
             tc.tile_pool(name="tmp", bufs=8) as tmp:

            # ---- persistent tiles ----
            # weights split into 4 k-group tiles; group 0 first on the sync
            # queue, the rest stream behind chunk 0's spikes
            KG = 4
            wsbk = [wpool.tile([128, KG * 2 * 128], mybir.dt.float16,
                               tag="wsb0", name="wsb0")]
            wsbr = wpool.tile([128, 3 * KG * 2 * 128], mybir.dt.float16,
                              tag="wsbr", name="wsbr")
            wsbk += [wsbr[:, (g - 1) * KG * 2 * 128:g * KG * 2 * 128]
                     for g in range(1, KT // KG)]
            rsb = wpool.tile([32, 2 * sum(TCS) * B], mybir.dt.float8e4,
                             tag="rsb")
            roff = {tc_: 2 * B * sum(t for t in TCS if t < tc_)
                    for tc_ in TCS}
            csb = wpool.tile([32, 2 * 2 * 128], mybir.dt.float8e4, tag="csb")
            # nT double-buffered by step parity: SACC(t) writes nT[(t+1)%2]
            # while LIF_V(t) still reads nT[t%2] -> no WAR, SACC can run early
            nTs = [state.tile([128, 2], _F32, tag=f"nT{i}", name=f"nT{i}")
                   for i in range(2)]
            Pt = state.tile([128, 2], _F32, tag="Pt")
            nc.vector.memset(Pt[:], 0.0)
            vst = [state.tile([128, 128], _F32, tag=f"v{i}", name=f"v{i}")
                   for i in range(2)]
            ist = [state.tile([128, 128], _F32, tag=f"i{i}", name=f"isyn{i}")
                   for i in range(4)]
            nc.vector.memset(vst[0][:], 0.0)
            nc.vector.memset(ist[0][:], 0.0)

            # prefetch spikes for chunks 0-1 ahead of weight groups 1-3 on
            # the sync queue (DMA order = issue order; chunk 0's k>=4
            # matmuls wait briefly but the spike stream never starves)
            def spk_dma(c, t0):
                TC = CHUNKS[c]
                spk_t = spkp.tile([128, KT * TC * B], mybir.dt.float8e4,
                                  tag="spk", name=f"spk_c{c}")
                nc.sync.dma_start(
                    spk_t[:].rearrange("p (k n) -> p k n", k=KT),
                    spk8.rearrange("(k p) n -> p k n",
                                   k=KT)[:, :, B * t0:B * t0 + TC * B])
                return spk_t

            # single sync-queue order (DMA_ENGINES is serial, order = issue
            # order): w0 s0 w-rest s1 rider/threshold s2 ...
            nc.sync.dma_start(wsbk[0][:], wgt[:, 0:KG * 2 * 128])
            # chunk 0's spikes split k0-3 / k4-15 so the very first matmul
            # waits only a quarter of the chunk-0 spike transfer
            TC0 = CHUNKS[0]
            s0A = spkp.tile([128, 4 * TC0 * B], mybir.dt.float8e4,
                            tag="spkA", name="spkA")
            nc.sync.dma_start(
                s0A[:].rearrange("p (k n) -> p k n", k=4),
                spk8.rearrange("(k p) n -> p k n", k=KT)[:, 0:4, 0:TC0 * B])
            s0B = spkp.tile([128, 12 * TC0 * B], mybir.dt.float8e4,
                            tag="spkB", name="spkB")
            nc.sync.dma_start(
                s0B[:].rearrange("p (k n) -> p k n", k=12),
                spk8.rearrange("(k p) n -> p k n", k=KT)[:, 4:16, 0:TC0 * B])
            pre_spk = [None]
            nc.sync.dma_start(wsbr[:], wgt[:, KG * 2 * 128:])
            pre_spk.append(spk_dma(1, CHUNKS[0]))
            nc.sync.dma_start(rsb[:], rid[:])
            nc.sync.dma_start(csb[:], crd[:])
            nc.sync.dma_start(nTs[0][:], nt0[:])

            # PE warmup: dummy matmuls on memset garbage keep the PE busy
            # from ~0.7us so the p-state ramp completes before real work;
            # they write a psume-bank tile that chunk 0 resets afterwards
            dumw = wpool.tile([128, 128], mybir.dt.float16, tag="dumw")
            nc.gpsimd.memset(dumw[:], 0.0)
            dumr = wpool.tile([128, 512], mybir.dt.float16, tag="dumr")
            nc.gpsimd.memset(dumr[:], 0.0)
            dps = psume.tile([128, 256], _F32, tag="pse0", name="dumps")
            for _ in range(28):
                nc.tensor.matmul(dps[:], dumw[:], dumr[:, 0:256],
                                 start=True, stop=True)

            # global step -> (w3 view, local index); filled as evacs are
            # emitted one chunk ahead of the scan
            w3map = []
            accs = []       # (acc tile, chunk start, TC) per chunk
            cstart = [0]
            for tc_ in CHUNKS:
                cstart.append(cstart[-1] + tc_)

            def emit_mm_evac(c):
                TC = CHUNKS[c]
                t0 = cstart[c]
                spk_t = pre_spk[c] if c < len(pre_spk) else spk_dma(c, t0)
                pss = []
                for (ta, tb) in _tblocks(TC):
                    nb = tb - ta
                    # tiny end-chunks use a dedicated PSUM bank so they
                    # don't WAR against the main double-buffer rotation
                    pool = psum if TC > 4 else psume
                    ps = [pool.tile([128, nb * B], _F32, tag=f"ps{ht}"
                                    if TC > 4 else f"pse{ht}",
                                    name=f"ps{c}_{ta}_{ht}")
                          for ht in range(2)]
                    pss.append(ps)
                    for k in range(KT):
                        for ht in range(2):
                            lhsT = wsbk[k // KG][:, ((k % KG) * 2 + ht) * 128:
                                                 ((k % KG) * 2 + ht + 1) * 128]
                            if c == 0 and k < 4:
                                rhs = s0A[:, k * TC * B + ta * B:
                                          k * TC * B + tb * B]
                            elif c == 0:
                                rhs = s0B[:, (k - 4) * TC * B + ta * B:
                                          (k - 4) * TC * B + tb * B]
                            else:
                                rhs = spk_t[:, k * TC * B + ta * B:
                                            k * TC * B + tb * B]
                            nc.tensor.matmul(
                                ps[ht][:], lhsT, rhs,
                                start=(k == 0), stop=False)
                    # DC-correction rider (fp8 DoubleRow) carries the stop
                    r8 = rsb[:, roff[TC]:roff[TC] + 2 * TC * B].rearrange(
                        "p (ko n) -> p ko n", ko=2)[:, :, ta * B:tb * B]
                    for ht in range(2):
                        l8 = csb[:, ht * 256:(ht + 1) * 256].rearrange(
                            "p (ko h) -> p ko h", ko=2)
                        nc.tensor.matmul(
                            ps[ht][:], l8, r8,
                            start=False, stop=True,
                            perf_mode=mybir.MatmulPerfMode.DoubleRow)
                # ---- evacuate PSUM -> SBUF (scalar engine) ----
                with tc.high_priority():
                    for bi, (ta, tb) in enumerate(_tblocks(TC)):
                        nb = tb - ta
                        wt = wev.tile([128, 2 * nb * B], _F32, tag="wt",
                                      name=f"wt{c}_{ta}")
                        for ht in range(2):
                            nc.scalar.activation(
                                wt[:, ht * nb * B:(ht + 1) * nb * B],
                                pss[bi][ht][:],
                                mybir.ActivationFunctionType.Copy,
                                bias=0.0, scale=float(2.0 ** -13))
                        w3 = wt[:].rearrange("p (g t b) -> p g t b",
                                             g=2, b=B)
                        for tl in range(ta, tb):
                            w3map.append((w3, tl - ta))
                accs.append(accp.tile([128, TC * 128], odt, tag="acc",
                                      name=f"acc{c}"))

            def emit_isyn(t):
                i3o = ist[t % 4][:].rearrange("p (h b) -> p h b", h=2)
                i3n = ist[(t + 1) % 4][:].rearrange("p (h b) -> p h b", h=2)
                w3, j = w3map[t]
                nc.vector.scalar_tensor_tensor(
                    i3n, i3o, a_syn, w3[:, :, j, :],
                    op0=_ALU.mult, op1=_ALU.add)

            def emit_scan_chunk(c):
                # per-step order [S0,S1,P,SACC,V0,V1,TSP(t+2)]: the i_syn
                # STT for step t+2 is emitted at step t (possibly into the
                # next chunk's just-evacuated weights), so every sem edge
                # hides under independent engine work
                TC = CHUNKS[c]
                t0 = cstart[c]
                acc = accs[c]
                if c == 0:
                    emit_isyn(0)
                    emit_isyn(1)
                for tl in range(TC):
                    t = t0 + tl
                    inew = ist[(t + 1) % 4]
                    vold, vnew = vst[t % 2], vst[(t + 1) % 2]
                    nTc, nTn = nTs[t % 2], nTs[(t + 1) % 2]
                    rs = tmp.tile([128, 2], _F32, tag="rs")
                    for ht in range(2):
                        sl = slice(ht * B, (ht + 1) * B)
                        s_out = acc[:, tl * 128 + ht * B:
                                    tl * 128 + (ht + 1) * B]
                        nc.vector._custom_dve(
                            LIF_S, out=s_out, in0=inew[:, sl],
                            in1=vold[:, sl],
                            s0=a_mem, s1=nTc[:, ht:ht + 1],
                            accum_out=rs[:, ht:ht + 1])
                    if t == T - 1:
                        continue  # v'/threshold of the last step are unused
                    # threshold: P += (c*0.99^-t)*rs ; nT' = 0.99^t*P+nT+Rd
                    nc.vector.scalar_tensor_tensor(
                        Pt[:], rs[:], sc_p[t], Pt[:],
                        op0=_ALU.mult, op1=_ALU.add)
                    nc.vector._custom_dve(
                        SACC, out=nTn[:], in0=Pt[:], in1=nTc[:],
                        s0=sc_s[t], imm2=rd[t])
                    for ht in range(2):
                        sl = slice(ht * B, (ht + 1) * B)
                        nc.vector._custom_dve(
                            LIF_V, out=vnew[:, sl], in0=inew[:, sl],
                            in1=vold[:, sl], s0=a_mem, s1=nTc[:, ht:ht + 1])
                    if t + 2 < len(w3map):
                        emit_isyn(t + 2)

            def emit_out(c):
                TC = CHUNKS[c]
                t0 = cstart[c]
                if c == NCH - 1:
                    # final chunk: per-step DMAs so the first piece's queue
                    # setup overlaps the last scan step
                    for tl in range(TC):
                        nc.sync.dma_start(
                            out[:, (t0 + tl) * 128:(t0 + tl + 1) * 128],
                            accs[c][:, tl * 128:(tl + 1) * 128])
                else:
                    nc.sync.dma_start(out[:, t0 * 128:(t0 + TC) * 128],
                                      accs[c][:])

            # chunk-level software pipeline: mm+evac(c) before scan(c-1);
            # out-DMA(c-1) emitted after evac(c+1) so it never delays an
            # evacuation in the in-order Act queue
            emit_mm_evac(0)
            for c in range(NCH):
                if c + 1 < NCH:
                    emit_mm_evac(c + 1)
                if c >= 3:
                    emit_out(c - 3)
                emit_scan_chunk(c)
            for c in range(NCH - 3, NCH):
                emit_out(c)
    nc.compile()
    return nc


_CACHE = {}


def _get_nc(a_mem, a_syn, lr, tgt):
    key = (tuple(CHUNKS), float(a_mem), float(a_syn), float(lr), float(tgt))
    if key not in _CACHE:
        _CACHE[key] = _build(a_mem, a_syn, lr, tgt)
    return _CACHE[key]


def kernel(input_spikes, weight, synaptic_strength, threshold,
           tau_mem, tau_syn, target_rate, homeostatic_lr):
    import ml_dtypes
    spikes = np.asarray(input_spikes, dtype=np.float32)
    w_eff = (np.asarray(weight, dtype=np.float32)
             * np.asarray(synaptic_strength, dtype=np.float32))
    thr = np.asarray(threshold, dtype=np.float32)
    tau_m = np.float32(tau_mem)
    tau_s = np.float32(tau_syn)
    tgt = np.float32(target_rate)
    lr = np.float32(homeostatic_lr)
    a_mem = np.float32(np.exp(np.float64(np.float32(-DT) / tau_m)))
    a_syn = np.float32(np.exp(np.float64(np.float32(-DT) / tau_s)))

    nc = _get_nc(a_mem, a_syn, lr, tgt)

    # quantize weights; zero fp16 subnormals so host E matches device exactly
    w16 = w_eff.astype(np.float16)
    w16[np.abs(w16.astype(np.float32)) < np.float32(2.0 ** -14)] = \
        np.float16(0.0)
    E = w16.astype(np.float32) - w_eff                      # [I, H]
    sbar = spikes.mean(axis=2)                              # [B, I], exact
    c_full = -(sbar @ E)                                    # [B, H] fp32

    # spikes [B,I,T] -> [I, T*B] chunk-blocked, TIME-MAJOR inside chunks
    sIT = spikes.transpose(1, 0, 2)      # [I, B, T]
    pieces = []
    t0 = 0
    for tc_ in CHUNKS:
        pieces.append(np.ascontiguousarray(
            sIT[:, :, t0:t0 + tc_].transpose(0, 2, 1)).reshape(I, tc_ * B))
        t0 += tc_
    spk8_prep = np.ascontiguousarray(
        np.concatenate(pieces, axis=1)).astype(ml_dtypes.float8_e4m3)

    # rider moving data (DoubleRow, k=64 as 32x2): per distinct TC a
    # [32, (ko, t*B+b)] block with rid[p, ko, t*B+b] = (ko*32+p == b)
    piecesd = []
    eye = np.eye(B, dtype=np.float32)            # [b', b]
    for tc_ in sorted(set(CHUNKS)):
        pat = np.kron(np.ones((1, tc_), np.float32), eye)   # [64, tc*B]
        piecesd.append(pat.reshape(2, 32, tc_ * B).transpose(
            1, 0, 2).reshape(32, 2 * tc_ * B))
    rid_prep = np.ascontiguousarray(
        np.concatenate(piecesd, axis=1)).astype(ml_dtypes.float8_e4m3)

    WSC = np.float32(2.0 ** 13)   # psum scale (descaled in the evacuation)
    in_maps = []
    for core in range(NCORES):
        shard16 = w16[:, core * HL:(core + 1) * HL]          # [I, 256] fp16
        wk = (shard16.astype(np.float32) * WSC).astype(np.float16)
        wk = wk.reshape(KT, 128, 2, 128).transpose(0, 2, 1, 3)
        wk = np.ascontiguousarray(wk.transpose(2, 0, 1, 3)
                                  ).reshape(128, KT * 2 * 128)  # [i,(k,ht,h)]
        nt0 = np.ascontiguousarray(
            -thr[core * HL:(core + 1) * HL].reshape(2, 128).T)
        cs = c_full[:, core * HL:(core + 1) * HL] * WSC      # [64, 256]
        # [b=ko*32+p, (ht,h)] -> [32, (ht, ko, h)]
        c8 = cs.reshape(2, 32, 2, 128).transpose(1, 2, 0, 3).reshape(32, 512)
        im = {"wgt": wk, "nt0": nt0, "spk8": spk8_prep,
              "crd": np.ascontiguousarray(c8.astype(ml_dtypes.float8_e4m3)),
              "rid": rid_prep}
        in_maps.append(im)

    res = run_bass_kernel_spmd(nc, in_maps, core_ids=list(range(NCORES)),
                               trace=False)
    kernel.last_result = res

    outs = []
    for core in range(NCORES):
        o = res.results[core]["out"].astype(np.float32).reshape(128, T, 2, B)
        outs.append(o.transpose(3, 2, 0, 1).reshape(B, HL, T))
    return np.ascontiguousarray(np.concatenate(outs, axis=1))
